# revision 47
# baseline (speedup 1.0000x reference)
"""BEVEncoder Trainium2 kernel: 8-core SPMD over 40000 BEV queries.

Self-contained: takes FULL inputs, shards queries across 8 NeuronCores,
runs projection+bilinear sampling (host-computed gather indices/weights,
device gather + PE weighted-reduce) and the 6-layer per-token FFN stack
on device, returns the FULL (1, 256, 200, 200) output.
"""
import numpy as np

BEV_H = 200
BEV_W = 200
EMBED = 256
N_LAYERS = 6
D_FF = 4 * EMBED
N_CAMS = 6
H_FEAT = 32
W_FEAT = 88
LN_EPS = 1e-5

N_CORES = 8
HW = BEV_H * BEV_W                  # 40000
TOK = 5120                          # padded tokens per core (40 * 128)
TOK_REAL = HW // N_CORES            # 5000
NWIN = TOK // 128                   # 40 windows of 128 points
CG = 8                              # gather groups per chunk (1MB per gather)

_CACHE = {}


# --------------------------------------------------------------------------
# Host-side projection + gather-plan construction
# --------------------------------------------------------------------------

def _projection(camera_intrinsics, camera_extrinsics, ref_pts):
    K = np.asarray(camera_intrinsics, np.float32)[0]      # (6,3,3)
    E = np.asarray(camera_extrinsics, np.float32)[0]      # (6,4,4)
    ref = np.asarray(ref_pts, np.float32)                 # (HW,3)
    R = E[:, :3, :3]
    t = E[:, :3, 3]
    R_e2c = np.swapaxes(R, -1, -2)
    t_e2c = -np.einsum('nij,nj->ni', R_e2c, t)
    pts_cam = np.einsum('nij,pj->npi', R_e2c, ref) + t_e2c[:, None, :]
    pts_img = np.einsum('nij,npj->npi', K, pts_cam)
    z = np.maximum(pts_img[..., 2], np.float32(1e-5))
    # px = ((u_norm+1)*W - 1)/2 with u_norm = 2u/W - 1  ==  u - 0.5
    px = pts_img[..., 0] / z - np.float32(0.5)            # (6, HW)
    py = pts_img[..., 1] / z - np.float32(0.5)
    return px, py


def _build_rows(px, py):
    """Per (cam, point, yrow) gather rows.

    Returns flat arrays sorted by point index:
      pt (int32), idx (int16, units of 256 fp16 elems into the feature
      table), wA, wB (float32: weights of columns s and s+1 of the 512-elem
      gathered pair)."""
    cams, P = px.shape
    x0 = np.floor(px)
    y0 = np.floor(py)
    fx = (px - x0).astype(np.float32)
    fy = (py - y0).astype(np.float32)
    x0 = x0.astype(np.int64)
    y0 = y0.astype(np.int64)

    pts_l, idx_l, wa_l, wb_l = [], [], [], []
    cam_ix = np.arange(cams)[:, None]
    pt_ix = np.broadcast_to(np.arange(P)[None, :], (cams, P))
    x_ok = (x0 >= -1) & (x0 <= W_FEAT - 1)
    s = np.clip(x0, 0, W_FEAT - 2)
    # weight of column s / s+1 given the x taps (x0 with 1-fx, x0+1 with fx)
    wcol_s = np.where(s == x0, 1.0 - fx, np.where(s == x0 + 1, fx, 0.0))
    wcol_s1 = np.where(s + 1 == x0, 1.0 - fx, np.where(s + 1 == x0 + 1, fx, 0.0))
    # x-tap validity folded in: tap x0 valid iff 0<=x0<=W-1; tap x0+1 iff <=W-1
    v_x0 = (x0 >= 0) & (x0 <= W_FEAT - 1)
    v_x1 = (x0 + 1 >= 0) & (x0 + 1 <= W_FEAT - 1)
    wcol_s = np.where((s == x0) & ~v_x0, 0.0, wcol_s)
    wcol_s = np.where((s == x0 + 1) & ~v_x1, 0.0, wcol_s)
    wcol_s1 = np.where((s + 1 == x0) & ~v_x0, 0.0, wcol_s1)
    wcol_s1 = np.where((s + 1 == x0 + 1) & ~v_x1, 0.0, wcol_s1)

    for yi, wy in ((y0, 1.0 - fy), (y0 + 1, fy)):
        y_ok = (yi >= 0) & (yi <= H_FEAT - 1)
        m = x_ok & y_ok & (wy > 0)
        if not m.any():
            continue
        row = (cam_ix * H_FEAT + yi) * W_FEAT + s          # (6, P)
        w_row = (wy / cams).astype(np.float32)
        pts_l.append(pt_ix[m])
        idx_l.append(row[m])
        wa_l.append((wcol_s * w_row)[m])
        wb_l.append((wcol_s1 * w_row)[m])

    pt = np.concatenate(pts_l) if pts_l else np.zeros(0, np.int64)
    idx = np.concatenate(idx_l) if idx_l else np.zeros(0, np.int64)
    wa = np.concatenate(wa_l) if wa_l else np.zeros(0, np.float32)
    wb = np.concatenate(wb_l) if wb_l else np.zeros(0, np.float32)
    keep = (np.abs(wa) > 0) | (np.abs(wb) > 0)
    pt, idx, wa, wb = pt[keep], idx[keep], wa[keep], wb[keep]
    order = np.argsort(pt, kind='stable')
    return pt[order], idx[order], wa[order].astype(np.float32), wb[order].astype(np.float32)


def _pack_cores(pt, idx, wa, wb):
    """Split rows by core, pack into 128-row groups within 128-point windows.

    Returns group_map [(window, s, first, last)], and per-core
    (idx_all (NG*128,) int16, waf (NG*128,128) f16, wbf)."""
    core_rows = []
    counts = np.zeros((N_CORES, NWIN), np.int64)
    for c in range(N_CORES):
        lo, hi = c * TOK_REAL, (c + 1) * TOK_REAL
        m = (pt >= lo) & (pt < hi)
        ptl = pt[m] - lo
        core_rows.append((ptl, idx[m], wa[m], wb[m]))
        counts[c] = np.bincount(ptl // 128, minlength=NWIN)

    S = np.maximum(1, np.ceil(counts / 128.0).astype(np.int64).max(axis=0))  # (NWIN,)
    g_base = np.concatenate([[0], np.cumsum(S)])[:-1]       # group offset per window
    NG = int(S.sum())
    group_map = []
    for w in range(NWIN):
        for s in range(int(S[w])):
            group_map.append((w, s, s == 0, s == int(S[w]) - 1))

    per_core = []
    for c in range(N_CORES):
        ptl, idxc, wac, wbc = core_rows[c]
        win = ptl // 128
        # ordinal within window (rows already sorted by point => by window)
        start = np.concatenate([[0], np.cumsum(np.bincount(win, minlength=NWIN))])[:-1]
        ordinal = np.arange(len(ptl)) - start[win]
        slot = g_base[win] * 128 + ordinal                  # destination row slot
        idx_all = np.zeros(NG * 128, np.int16)
        widx = np.zeros(NG * 128, np.float32)
        wsa = np.zeros(NG * 128, np.float32)
        wsb = np.zeros(NG * 128, np.float32)
        idx_all[slot] = idxc.astype(np.int16)
        widx[slot] = (ptl % 128).astype(np.float32)
        wsa[slot] = wac
        wsb[slot] = wbc
        per_core.append((idx_all, widx, wsa, wsb))
    return NG, group_map, per_core


# --------------------------------------------------------------------------
# Device kernel builder
# --------------------------------------------------------------------------

def _build_kernel(NG, group_map, flags, debug=False, n_layers=N_LAYERS):
    import concourse.bacc as bacc
    import concourse.tile as tile
    from concourse import mybir

    F32 = mybir.dt.float32
    F16 = mybir.dt.float16
    I16 = mybir.dt.int16
    AT = mybir.ActivationFunctionType
    OP = mybir.AluOpType

    has_sca_b, has_b2, has_g1, has_g2, has_b1 = flags
    nchunks = (NG + CG - 1) // CG
    NROW = 16896  # feature-table rows (cam*32+y)*88+x

    nc = bacc.Bacc("TRN2", target_bir_lowering=False, debug=False,
                   num_devices=N_CORES)

    d_tbl = nc.dram_tensor("tbl", [NROW * EMBED], F16, kind="ExternalInput")
    d_idx = nc.dram_tensor("idx", [32, NG * 8], I16, kind="ExternalInput")
    d_widx = nc.dram_tensor("widx", [128, NG], F32, kind="ExternalInput")
    d_wsa = nc.dram_tensor("wsa", [128, NG], F32, kind="ExternalInput")
    d_wsb = nc.dram_tensor("wsb", [128, NG], F32, kind="ExternalInput")
    d_iota = nc.dram_tensor("iota", [128, 128], F16, kind="ExternalInput")
    d_x0 = nc.dram_tensor("x0", [128, NWIN, EMBED], F16, kind="ExternalInput")
    d_scaw = nc.dram_tensor("scaw", [N_LAYERS, 128, 2, EMBED], F16, kind="ExternalInput")
    d_w1 = nc.dram_tensor("w1", [N_LAYERS, 128, 16, 128], F16, kind="ExternalInput")
    d_w2 = nc.dram_tensor("w2", [N_LAYERS, 128, 8, EMBED], F16, kind="ExternalInput")
    d_b1 = nc.dram_tensor("b1", [N_LAYERS, 128, 8], F32, kind="ExternalInput")
    d_eye = nc.dram_tensor("eye", [128, 128], F16, kind="ExternalInput")
    if has_sca_b:
        d_scab = nc.dram_tensor("scab", [N_LAYERS, 128, 2, EMBED], F16, kind="ExternalInput")
    if has_b2:
        d_b2t = nc.dram_tensor("b2t", [N_LAYERS, 128, 2, EMBED], F16, kind="ExternalInput")
    if has_g1:
        d_g1 = nc.dram_tensor("g1t", [N_LAYERS, 128, EMBED], F16, kind="ExternalInput")
        d_bt1 = nc.dram_tensor("bt1t", [N_LAYERS, 128, EMBED], F16, kind="ExternalInput")
    if has_g2:
        d_g2 = nc.dram_tensor("g2t", [N_LAYERS, 128, EMBED], F16, kind="ExternalInput")
        d_bt2 = nc.dram_tensor("bt2t", [N_LAYERS, 128, EMBED], F16, kind="ExternalInput")
    d_out = nc.dram_tensor("xout", [128, NWIN, EMBED], F32, kind="ExternalOutput")
    if debug:
        d_smp = nc.dram_tensor("smpdump", [128, 2, TOK], F16, kind="ExternalOutput")
        d_a = nc.dram_tensor("adump", [128, NWIN, EMBED], F16, kind="ExternalOutput")
        d_x1 = nc.dram_tensor("x1dump", [128, NWIN, EMBED], F16, kind="ExternalOutput")
        d_ht = nc.dram_tensor("htdump", [128, 8, EMBED], F16, kind="ExternalOutput")
        d_a2 = nc.dram_tensor("a2dump", [128, NWIN, EMBED], F16, kind="ExternalOutput")

    import concourse.bass as bass

    # last group index of each psum bank (bank = 4 windows = 512 points)
    last_group_of_bank = {}
    for g, (w, s, first, last) in enumerate(group_map):
        if last and (w % 4 == 3 or w == NWIN - 1):
            last_group_of_bank[g] = w // 4

    with tile.TileContext(nc) as tc:
        with (
            tc.tile_pool(name="persist", bufs=1) as pp,
            tc.tile_pool(name="stats", bufs=1) as stp,
        ):
            x = pp.tile([128, NWIN, EMBED], F16, tag="x")
            a = pp.tile([128, NWIN, EMBED], F16, tag="a")
            x1 = pp.tile([128, NWIN, EMBED], F16, tag="x1")
            x1t = pp.tile([128, NWIN, 2, 128], F16, tag="x1t")
            smp_lo = pp.tile([128, TOK], F16, tag="smplo")
            smp_hi = pp.tile([128, TOK], F16, tag="smphi")
            idxs = pp.tile([32, NG * 8], I16, tag="idxs")
            eye = pp.tile([128, 128], F16, tag="eye")
            iota = pp.tile([128, 128], F16, tag="iota")
            widx = pp.tile([128, NG], F32, tag="widx")
            wsa = pp.tile([128, NG], F32, tag="wsa")
            wsb = pp.tile([128, NG], F32, tag="wsb")
            epst = pp.tile([128, 1], F32, tag="epst")

            st6 = stp.tile([128, NWIN, 6], F32, tag="st6")
            mv = stp.tile([128, NWIN, 2], F32, tag="mv")
            sct = stp.tile([128, NWIN, 1], F32, tag="sct")
            rst = stp.tile([128, NWIN, 1], F32, tag="rst")
            taut = stp.tile([128, NWIN, 1], F32, tag="taut")
            scr1 = stp.tile([128, NWIN, 1], F32, tag="scr1")
            scr2 = stp.tile([128, NWIN, 1], F32, tag="scr2")

            nc.sync.dma_start(x[:], d_x0.ap())
            nc.sync.dma_start(idxs[:], d_idx.ap())
            nc.sync.dma_start(eye[:], d_eye.ap())
            nc.sync.dma_start(iota[:], d_iota.ap())
            nc.sync.dma_start(widx[:], d_widx.ap())
            nc.sync.dma_start(wsa[:], d_wsa.ap())
            nc.sync.dma_start(wsb[:], d_wsb.ap())
            nc.vector.memset(epst[:], LN_EPS)

            # ---------------- Stage 1: gather + weighted reduce ----------
            src_ap = bass.AP(d_tbl, 0, [[256, NROW - 1], [1, 512]])
            with (
                tc.tile_pool(name="gat", bufs=3) as gp,
                tc.tile_pool(name="scl", bufs=6) as scp,
                tc.tile_pool(name="smps", bufs=2, space="PSUM") as smpp,
            ):
                ps_lo = None
                for ch in range(nchunks):
                    g0 = ch * CG
                    cg = min(CG, NG - g0)
                    gat = gp.tile([128, CG, 512], F16, tag="gat")
                    nc.gpsimd.dma_gather(
                        gat[:, 0:cg, :], src_ap,
                        idxs[:, g0 * 8:(g0 + cg) * 8],
                        num_idxs=cg * 128, num_idxs_reg=cg * 128,
                        elem_size=512, elem_step=256,
                    )
                    for gl in range(cg):
                        g = g0 + gl
                        w, s, first, last = group_map[g]
                        if w % 4 == 0 and first:
                            ps_lo = smpp.tile([128, 512], F32, tag="pslo")
                            ps_hi = smpp.tile([128, 512], F32, tag="pshi")
                        col = 128 * (w % 4)
                        lo = ps_lo[:, col:col + 128]
                        hi = ps_hi[:, col:col + 128]
                        selA = scp.tile([128, 128], F16, tag="selA")
                        nc.vector.tensor_scalar(
                            selA[:], iota[:], widx[:, g:g + 1], wsa[:, g:g + 1],
                            op0=OP.is_equal, op1=OP.mult)
                        selB = scp.tile([128, 128], F16, tag="selB")
                        nc.vector.tensor_scalar(
                            selB[:], iota[:], widx[:, g:g + 1], wsb[:, g:g + 1],
                            op0=OP.is_equal, op1=OP.mult)
                        nc.tensor.matmul(lo, gat[:, gl, 0:128], selA[:],
                                         start=first, stop=False)
                        nc.tensor.matmul(lo, gat[:, gl, 256:384], selB[:],
                                         start=False, stop=last)
                        nc.tensor.matmul(hi, gat[:, gl, 128:256], selA[:],
                                         start=first, stop=False)
                        nc.tensor.matmul(hi, gat[:, gl, 384:512], selB[:],
                                         start=False, stop=last)
                        if g in last_group_of_bank:
                            b = last_group_of_bank[g]
                            pts0 = b * 512
                            width = min(512, TOK - pts0)
                            nc.scalar.copy(smp_lo[:, pts0:pts0 + width],
                                           ps_lo[:, 0:width])
                            nc.scalar.copy(smp_hi[:, pts0:pts0 + width],
                                           ps_hi[:, 0:width])

            # ---------------- Stage 2: 6-layer FFN stack -----------------
            if debug:
                nc.sync.dma_start(d_smp.ap()[:, 0, :], smp_lo[:])
                nc.sync.dma_start(d_smp.ap()[:, 1, :], smp_hi[:])
            with (
                tc.tile_pool(name="lw", bufs=2) as lwp,
                tc.tile_pool(name="ht", bufs=3) as hp,
                tc.tile_pool(name="xo", bufs=4) as xop,
                tc.tile_pool(name="ps_sm", bufs=3, space="PSUM") as psmall,
                tc.tile_pool(name="ps_h", bufs=2, space="PSUM") as psh,
                tc.tile_pool(name="ps_tr", bufs=1, space="PSUM") as pstr,
            ):

                def ln_finalize(c0, c1):
                    # combine the two 128-halves from bn_stats 6-vectors for
                    # windows [c0, c1): mean = (m1+m2)/2,
                    # var = (M2a+M2b)/256 + ((m1-m2)/2)^2, then rs and tau.
                    m1 = st6[:, c0:c1, 1:2]; m2 = st6[:, c0:c1, 4:5]
                    M2a = st6[:, c0:c1, 2:3]; M2b = st6[:, c0:c1, 5:6]
                    s1_ = scr1[:, c0:c1, :]; s2_ = scr2[:, c0:c1, :]
                    nc.vector.tensor_tensor(s1_, m1, m2, op=OP.subtract)
                    nc.vector.tensor_scalar(s1_, s1_, 0.5, None, op0=OP.mult)
                    nc.vector.tensor_tensor(s1_, s1_, s1_, op=OP.mult)
                    nc.vector.tensor_tensor(s2_, M2a, M2b, op=OP.add)
                    nc.vector.scalar_tensor_tensor(
                        mv[:, c0:c1, 1:2], s2_, 1.0 / 256.0, s1_,
                        op0=OP.mult, op1=OP.add)
                    nc.vector.tensor_tensor(s2_, m1, m2, op=OP.add)
                    nc.vector.tensor_scalar(mv[:, c0:c1, 0:1], s2_, 0.5, None,
                                            op0=OP.mult)
                    nc.scalar.activation(sct[:, c0:c1, 0], mv[:, c0:c1, 1],
                                         AT.Sqrt, bias=epst[:])
                    nc.vector.reciprocal(rst[:, c0:c1, 0], sct[:, c0:c1, 0])
                    nc.vector.tensor_tensor(taut[:, c0:c1, 0], mv[:, c0:c1, 0],
                                            rst[:, c0:c1, 0], op=OP.mult)

                LNCH = 20  # finalize chunk (windows)

                for l in range(n_layers):
                    scaw = lwp.tile([128, 2, EMBED], F16, tag="scaw")
                    w1t = lwp.tile([128, 16, 128], F16, tag="w1t")
                    w2t = lwp.tile([128, 8, EMBED], F16, tag="w2t")
                    b1t = lwp.tile([128, 8], F32, tag="b1t")
                    nc.sync.dma_start(scaw[:], d_scaw.ap()[l])
                    nc.sync.dma_start(w1t[:], d_w1.ap()[l])
                    nc.sync.dma_start(w2t[:], d_w2.ap()[l])
                    nc.sync.dma_start(b1t[:], d_b1.ap()[l])
                    if has_sca_b:
                        scab2 = lwp.tile([128, 2, EMBED], F16, tag="scab")
                        nc.sync.dma_start(scab2[:], d_scab.ap()[l])
                    if has_b2:
                        b2t2 = lwp.tile([128, 2, EMBED], F16, tag="b2t")
                        nc.sync.dma_start(b2t2[:], d_b2t.ap()[l])
                    if has_g1:
                        g1t = lwp.tile([128, EMBED], F16, tag="g1t")
                        bt1t = lwp.tile([128, EMBED], F16, tag="bt1t")
                        nc.sync.dma_start(g1t[:], d_g1.ap()[l])
                        nc.sync.dma_start(bt1t[:], d_bt1.ap()[l])
                    if has_g2:
                        g2t = lwp.tile([128, EMBED], F16, tag="g2t")
                        bt2t = lwp.tile([128, EMBED], F16, tag="bt2t")
                        nc.sync.dma_start(g2t[:], d_g2.ap()[l])
                        nc.sync.dma_start(bt2t[:], d_bt2.ap()[l])

                    # ---- sca + residual + LN1 stats (token-tile pairs) ----
                    for tp in range(NWIN // 2):
                        pss = psmall.tile([128, 2, EMBED], F32, tag="pss")
                        for tt in range(2):
                            g = 2 * tp + tt
                            nc.tensor.matmul(pss[:, tt, :],
                                             smp_lo[:, g * 128:(g + 1) * 128],
                                             scaw[:, 0, :], start=True, stop=False)
                            nc.tensor.matmul(pss[:, tt, :],
                                             smp_hi[:, g * 128:(g + 1) * 128],
                                             scaw[:, 1, :], start=False, stop=True)
                        ga = 2 * tp
                        nc.vector.scalar_tensor_tensor(
                            a[:, ga:ga + 2, :], pss[:], 1.0, x[:, ga:ga + 2, :],
                            op0=OP.mult, op1=OP.add)
                        if has_sca_b:
                            nc.vector.tensor_tensor(
                                a[:, ga:ga + 2, :], a[:, ga:ga + 2, :],
                                scab2[:], op=OP.add)
                        nc.vector.bn_stats(st6[:, ga, :], a[:, ga, :])
                        nc.vector.bn_stats(st6[:, ga + 1, :], a[:, ga + 1, :])

                    if debug and l == 0:
                        nc.sync.dma_start(d_a.ap(), a[:])
                    for c0 in range(0, NWIN, LNCH):
                        ln_finalize(c0, c0 + LNCH)

                    # ---- LN1 apply + transpose ----
                    for g in range(NWIN):
                        nc.vector.tensor_scalar(
                            x1[:, g, :], a[:, g, :],
                            rst[:, g, :], taut[:, g, :],
                            op0=OP.mult, op1=OP.subtract)
                        if has_g1:
                            nc.vector.tensor_tensor(x1[:, g, :], x1[:, g, :],
                                                    g1t[:], op=OP.mult)
                            nc.vector.tensor_tensor(x1[:, g, :], x1[:, g, :],
                                                    bt1t[:], op=OP.add)
                        ptr = pstr.tile([128, 2, 128], F16, tag="ptr")
                        nc.tensor.transpose(ptr[:, 0, :], x1[:, g, 0:128], eye[:])
                        nc.tensor.transpose(ptr[:, 1, :], x1[:, g, 128:256], eye[:])
                        nc.scalar.copy(x1t[:, g, :, :], ptr[:])

                    # ---- FFN + residual + LN2 stats ----
                    for tg in range(NWIN // 2):
                        ht = hp.tile([128, 8, EMBED], F16, tag="ht")
                        for half in range(2):
                            psh_t = psh.tile([128, 4, EMBED], F32, tag="psh")
                            for mq in range(4):
                                mb = half * 4 + mq
                                nc.tensor.matmul(
                                    psh_t[:, mq, :], w1t[:, mb, :],
                                    x1t[:, 2 * tg:2 * tg + 2, 0, :],
                                    start=True, stop=False)
                                nc.tensor.matmul(
                                    psh_t[:, mq, :], w1t[:, 8 + mb, :],
                                    x1t[:, 2 * tg:2 * tg + 2, 1, :],
                                    start=False, stop=True)
                            if has_b1:
                                for mq in range(4):
                                    mb = half * 4 + mq
                                    nc.scalar.activation(
                                        ht[:, mb, :], psh_t[:, mq, :],
                                        AT.Gelu, bias=b1t[:, mb:mb + 1])
                            else:
                                nc.scalar.activation(
                                    ht[:, half * 4:half * 4 + 4, :]
                                    .rearrange("p a b -> p (a b)"),
                                    psh_t[:].rearrange("p a b -> p (a b)"),
                                    AT.Gelu)
                        if debug and l == 0 and tg == 0:
                            nc.sync.dma_start(d_ht.ap(), ht[:])
                        pso = psmall.tile([128, 2, EMBED], F32, tag="pss")
                        for tt in range(2):
                            for kb in range(8):
                                nc.tensor.matmul(
                                    pso[:, tt, :],
                                    ht[:, kb, tt * 128:tt * 128 + 128],
                                    w2t[:, kb, :],
                                    start=(kb == 0), stop=(kb == 7))
                        ga = 2 * tg
                        nc.vector.scalar_tensor_tensor(
                            a[:, ga:ga + 2, :], pso[:], 1.0, x1[:, ga:ga + 2, :],
                            op0=OP.mult, op1=OP.add)
                        if has_b2:
                            nc.vector.tensor_tensor(
                                a[:, ga:ga + 2, :], a[:, ga:ga + 2, :],
                                b2t2[:], op=OP.add)
                        nc.vector.bn_stats(st6[:, ga, :], a[:, ga, :])
                        nc.vector.bn_stats(st6[:, ga + 1, :], a[:, ga + 1, :])

                    if debug and l == 0:
                        nc.sync.dma_start(d_x1.ap(), x1[:])
                        nc.sync.dma_start(d_a2.ap(), a[:])
                    for c0 in range(0, NWIN, LNCH):
                        ln_finalize(c0, c0 + LNCH)

                    # ---- LN2 apply -> x (or output) ----
                    for g in range(NWIN):
                        if l < n_layers - 1:
                            nc.vector.tensor_scalar(
                                x[:, g, :], a[:, g, :],
                                rst[:, g, :], taut[:, g, :],
                                op0=OP.mult, op1=OP.subtract)
                            if has_g2:
                                nc.vector.tensor_tensor(x[:, g, :], x[:, g, :],
                                                        g2t[:], op=OP.mult)
                                nc.vector.tensor_tensor(x[:, g, :], x[:, g, :],
                                                        bt2t[:], op=OP.add)
                        else:
                            xo = xop.tile([128, EMBED], F32, tag="xo")
                            nc.vector.tensor_scalar(
                                xo[:], a[:, g, :],
                                rst[:, g, :], taut[:, g, :],
                                op0=OP.mult, op1=OP.subtract)
                            if has_g2:
                                nc.vector.tensor_tensor(xo[:], xo[:], g2t[:],
                                                        op=OP.mult)
                                nc.vector.tensor_tensor(xo[:], xo[:], bt2t[:],
                                                        op=OP.add)
                            nc.sync.dma_start(d_out.ap()[:, g, :], xo[:])

    nc.compile()
    return nc


# --------------------------------------------------------------------------
# Cached PJRT runner (avoids per-call re-jit of bass2jax._body)
# --------------------------------------------------------------------------

class _Runner:
    def __init__(self, nc):
        import jax
        import numpy as _np
        from jax.sharding import Mesh, PartitionSpec
        from jax.experimental.shard_map import shard_map
        from concourse import bass2jax, mybir

        bass2jax.install_neuronx_cc_hook()
        self.nc = nc
        part_name = nc.partition_id_tensor.name if nc.partition_id_tensor else None
        in_names, out_names, out_avals, zero_shapes = [], [], [], []
        for alloc in nc.m.functions[0].allocations:
            if not isinstance(alloc, mybir.MemoryLocationSet):
                continue
            name = alloc.memorylocations[0].name
            if alloc.kind == "ExternalInput":
                if name != part_name:
                    in_names.append(name)
            elif alloc.kind == "ExternalOutput":
                shape = tuple(alloc.tensor_shape)
                dtype = mybir.dt.np(alloc.dtype)
                out_names.append(name)
                out_avals.append(jax.core.ShapedArray(shape, dtype))
                zero_shapes.append((shape, dtype))
        self.in_names = list(in_names)
        self.out_names = out_names
        self.out_avals = out_avals
        self.zero_shapes = zero_shapes
        n_params = len(in_names)
        all_in_names = list(in_names) + list(out_names)
        if part_name is not None:
            all_in_names.append(part_name)

        def _body(*args):
            operands = list(args)
            if part_name is not None:
                operands.append(bass2jax.partition_id_tensor())
            outs = bass2jax._bass_exec_p.bind(
                *operands,
                out_avals=tuple(out_avals),
                in_names=tuple(all_in_names),
                out_names=tuple(out_names),
                lowering_input_output_aliases=(),
                sim_require_finite=True,
                sim_require_nnan=True,
                nc=nc,
            )
            return tuple(outs)

        devices = jax.devices()[:N_CORES]
        self.mesh = Mesh(_np.asarray(devices), ("core",))
        specs = (PartitionSpec("core"),) * (n_params + len(out_names))
        out_specs = (PartitionSpec("core"),) * len(out_names)
        self.sharded = jax.jit(
            shard_map(_body, mesh=self.mesh, in_specs=specs,
                      out_specs=out_specs, check_rep=False),
            keep_unused=True,
        )

    def concat_inputs(self, in_maps):
        concat = [
            np.concatenate([np.asarray(m[n]) for m in in_maps], axis=0)
            for n in self.in_names
        ]
        zeros = [np.zeros((N_CORES * s[0], *s[1:]), d)
                 for (s, d) in self.zero_shapes]
        return concat + zeros

    def __call__(self, in_maps):
        args = self.concat_inputs(in_maps)
        out_arrs = self.sharded(*args)
        res = []
        for c in range(N_CORES):
            res.append({
                n: np.asarray(out_arrs[i]).reshape(
                    N_CORES, *self.out_avals[i].shape)[c]
                for i, n in enumerate(self.out_names)
            })
        return res


# --------------------------------------------------------------------------
# Entry point
# --------------------------------------------------------------------------

_LAST = {}


def kernel(image_feat, camera_intrinsics, camera_extrinsics, ref_pts,
           bev_emb, bev_pos, sca_W, sca_b, sca_g, sca_bt,
           ffn_W1, ffn_b1, ffn_W2, ffn_b2, ffn_g, ffn_bt):

    image_feat = np.asarray(image_feat, np.float32)
    sca_W = np.asarray(sca_W, np.float32)
    sca_b = np.asarray(sca_b, np.float32)
    sca_g = np.asarray(sca_g, np.float32)
    sca_bt = np.asarray(sca_bt, np.float32)
    ffn_W1 = np.asarray(ffn_W1, np.float32)
    ffn_b1 = np.asarray(ffn_b1, np.float32)
    ffn_W2 = np.asarray(ffn_W2, np.float32)
    ffn_b2 = np.asarray(ffn_b2, np.float32)
    ffn_g = np.asarray(ffn_g, np.float32)
    ffn_bt = np.asarray(ffn_bt, np.float32)

    px, py = _projection(camera_intrinsics, camera_extrinsics, ref_pts)
    pt, idx, wa, wb = _build_rows(px, py)
    NG, group_map, per_core = _pack_cores(pt, idx, wa, wb)

    has_sca_b = bool(np.any(sca_b != 0))
    has_b2 = bool(np.any(ffn_b2 != 0))
    has_g1 = bool(np.any(sca_g != 1) or np.any(sca_bt != 0))
    has_g2 = bool(np.any(ffn_g != 1) or np.any(ffn_bt != 0))
    has_b1 = bool(np.any(ffn_b1 != 0))
    flags = (has_sca_b, has_b2, has_g1, has_g2, has_b1)

    key = (NG, tuple(group_map), flags)
    if key not in _CACHE:
        _CACHE[key] = _Runner(_build_kernel(NG, group_map, flags))
    runner = _CACHE[key]

    # ---- shared input prep ----
    tbl = np.ascontiguousarray(
        image_feat.transpose(0, 2, 3, 1)).astype(np.float16).reshape(-1)
    x0_full = (np.asarray(bev_emb, np.float32)
               + np.asarray(bev_pos, np.float32)).astype(np.float16)
    scaw_h = np.ascontiguousarray(
        sca_W.reshape(N_LAYERS, 2, 128, EMBED).transpose(0, 2, 1, 3)
    ).astype(np.float16)
    w1_h = np.ascontiguousarray(
        ffn_W1.reshape(N_LAYERS, 2, 128, 8, 128).transpose(0, 2, 1, 3, 4)
        .reshape(N_LAYERS, 128, 16, 128)).astype(np.float16)
    w2_h = np.ascontiguousarray(
        ffn_W2.reshape(N_LAYERS, 8, 128, EMBED).transpose(0, 2, 1, 3)
    ).astype(np.float16)
    b1_h = np.ascontiguousarray(
        ffn_b1.reshape(N_LAYERS, 8, 128).transpose(0, 2, 1)).astype(np.float32)
    eye = np.eye(128, dtype=np.float16)

    def bcast(v):  # (L, E) -> (L, 128, E) broadcast tiles
        return np.ascontiguousarray(
            np.broadcast_to(v[:, None, :], (N_LAYERS, 128, EMBED))
        ).astype(np.float16)

    iota_h = np.broadcast_to(np.arange(128, dtype=np.float16)[None, :],
                             (128, 128)).copy()
    in_maps = []
    for c in range(N_CORES):
        idx_all, widx, wsa, wsb = per_core[c]
        wrapped = idx_all.reshape(-1, 16).T.copy()            # (16, NG*8)
        idx_in = np.concatenate([wrapped, wrapped], axis=0)   # (32, NG*8)
        x0c = np.zeros((TOK, EMBED), np.float16)
        x0c[:TOK_REAL] = x0_full[c * TOK_REAL:(c + 1) * TOK_REAL]
        x0c = np.ascontiguousarray(
            x0c.reshape(NWIN, 128, EMBED).transpose(1, 0, 2))
        m = {
            "tbl": tbl, "idx": idx_in,
            "widx": np.ascontiguousarray(widx.reshape(NG, 128).T),
            "wsa": np.ascontiguousarray(wsa.reshape(NG, 128).T),
            "wsb": np.ascontiguousarray(wsb.reshape(NG, 128).T),
            "iota": iota_h,
            "x0": x0c, "scaw": scaw_h, "w1": w1_h, "w2": w2_h,
            "b1": b1_h, "eye": eye,
        }
        if has_sca_b:
            m["scab"] = np.ascontiguousarray(
                np.broadcast_to(bcast(sca_b)[:, :, None, :],
                                (N_LAYERS, 128, 2, EMBED)))
        if has_b2:
            m["b2t"] = np.ascontiguousarray(
                np.broadcast_to(bcast(ffn_b2)[:, :, None, :],
                                (N_LAYERS, 128, 2, EMBED)))
        if has_g1:
            m["g1t"] = bcast(sca_g)
            m["bt1t"] = bcast(sca_bt)
        if has_g2:
            m["g2t"] = bcast(ffn_g)
            m["bt2t"] = bcast(ffn_bt)
        in_maps.append(m)

    results = runner(in_maps)
    _LAST["runner"] = runner
    _LAST["in_maps"] = in_maps

    out = np.empty((HW, EMBED), np.float32)
    for c in range(N_CORES):
        xo = results[c]["xout"]                               # (128, NWIN, E)
        out[c * TOK_REAL:(c + 1) * TOK_REAL] = (
            xo.transpose(1, 0, 2).reshape(TOK, EMBED)[:TOK_REAL])
    return np.ascontiguousarray(out.T).reshape(1, EMBED, BEV_H, BEV_W)
